# revision 1
# baseline (speedup 1.0000x reference)
"""Trainium2 Bass kernel for the bilinear block classifier.

logits[n, c] = sum_{k,i,j} W[c, k*4096+i*64+j] * head[n, 64k+i] * tail[n, 64k+j] + b[c]
head/tail [4096, 768] fp32, W [97, 49152] fp32, b [97] fp32.

Data-parallel over 8 NeuronCores (512 samples each). Per core, the 49152-dim
outer-product feature tensor is materialized chunk-by-chunk (384 chunks of
128 features x 512 samples) in [feature, sample] layout, then contracted
against host-reordered W^T chunks with fp32 PSUM accumulation into a single
[97, 512] bank.

Chunks are processed in groups to amortize per-instruction overhead:
  route A (PE-replication), groups of 2: K=64 selection matmuls broadcast two
    head^T rows across 128 partitions each -> one 2-bank fp32 PSUM tile; one
    ScalarE copy evacuates the pair to fp16 SBUF.
  route C (host-replication), groups of 4: the replicated head^T rows arrive
    pre-built from the host in one 512KB DMA.
One fp16 VectorE tensor-multiply per group (2x mode) against a host-built
4x-duplicated tail^T tile forms the outer products. Routes are interleaved
to balance PE / ScalarE / VectorE / DMA. Bias is added during the final
PSUM evacuation; the host reassembles [4096, 97] from per-core [97, 512].
"""

import numpy as np

EMB = 768
BLK = 64
NCLS = 97
NTOT = 4096
NB = 12             # feature blocks of 64
NCORES = 8
NPC = NTOT // NCORES    # 512 samples per core
NM = BLK // 2           # 32 chunks per block (2 i-rows x 64 j each)
NCHUNK = NB * NM        # 384 chunks of 128 features

GA = 2               # chunks per route-A group (PSUM banks per tile)
GC = 4               # chunks per route-C group
# per-k patterns of groups; alternate to average 4.5 C-groups per block
# "C" = 4-chunk host-replicated group, "c" = 2-chunk host-replicated group,
# "A" = 2-chunk PE-replicated group; 18 C-chunks + 14 A-chunks per block
K_PATTERNS = [
    ["C", "A", "C", "A", "C", "A", "C", "A", "C", "A", "A"],
]
N_WARMUP = 0        # PE warmup matmuls issued during the DMA head

_CACHE = {}


LAST_K_PATTERN = ["C", "A", "C", "A", "C", "A", "C", "A", "C", "A", "A"]


def _groups():
    """Yield (route, k, m0, size) for every group, in chunk order."""
    out = []
    for k in range(NB):
        pat = LAST_K_PATTERN if k == NB - 1 else K_PATTERNS[k % len(K_PATTERNS)]
        m0 = 0
        for r in pat:
            size = GC if r == "C" else GA
            out.append((r, k, m0, size))
            m0 += size
        assert m0 == NM
    return out


def _split_excess_waits(nc, limit=1):
    """walrus in this toolchain rejects instructions carrying more than
    `limit` semaphore waits; split extras into preceding wait-only Drains."""
    import concourse.mybir as mybir

    n_new = 0
    for bb in nc.main_func.blocks:
        new_list = []
        for ins in bb.instructions:
            si = ins.sync_info
            if si is not None and si.on_wait and len(si.on_wait) > limit:
                waits = list(si.on_wait)
                extra, keep = waits[:-limit], waits[-limit:]
                for i in range(0, len(extra), limit):
                    chunk = extra[i : i + limit]
                    n_new += 1
                    d = mybir.InstDrain(
                        name=f"I-waitsplit-{n_new}",
                        engine=ins.engine,
                        ins=[],
                        outs=[],
                        sync_info=mybir.SyncInfo(on_wait=chunk, on_update=[]),
                    )
                    nc.register_instruction(d)
                    new_list.append(d)
                si.on_wait = keep
            new_list.append(ins)
        bb.instructions[:] = new_list
    return n_new


def _build_nc():
    import concourse.bass as bass
    import concourse.mybir as mybir
    import concourse.tile as tile

    dt = mybir.dt
    nc = bass.Bass()

    groups = _groups()
    n_c4 = sum(1 for g in groups if g[0] == "C")
    n_c2 = sum(1 for g in groups if g[0] == "c")

    napk = sum(GA for r in K_PATTERNS[0] if r == "A")   # A-pairs per block
    nks = (NB + 3) // 4                                  # k-slots per base
    b1p = nc.dram_tensor(
        "b1p", [8, nks * napk * NPC], dt.float16, kind="ExternalInput"
    )
    b2d = nc.dram_tensor(
        "b2d", [128, NB * 2 * NPC], dt.float16, kind="ExternalInput"
    )
    wt = nc.dram_tensor("wt", [NB, 128, NM * NCLS], dt.float16, kind="ExternalInput")
    bia = nc.dram_tensor("bias", [NCLS, 1], dt.float32, kind="ExternalInput")
    s2 = nc.dram_tensor("s2", [128, 128], dt.float16, kind="ExternalInput")
    b1r = nc.dram_tensor(
        "b1r", [n_c4, 128, GC * NPC], dt.float16, kind="ExternalInput"
    )
    b1r2 = nc.dram_tensor(
        "b1r2", [max(n_c2, 1), 128, GA * NPC], dt.float16, kind="ExternalInput"
    )
    out = nc.dram_tensor("logits_t", [NCLS, NPC], dt.float32, kind="ExternalOutput")

    with tile.TileContext(nc) as tc:
        with (
            tc.tile_pool(name="cst", bufs=1) as cst,
            tc.tile_pool(name="wp", bufs=4) as wp,
            tc.tile_pool(name="r1a", bufs=5) as r1a,
            tc.tile_pool(name="r1c", bufs=7) as r1c,
            tc.tile_pool(name="blta", bufs=5) as blta,
            tc.tile_pool(name="bltc", bufs=7) as bltc,
            tc.tile_pool(name="ps", bufs=6, space="PSUM") as ps,
            tc.tile_pool(name="accp", bufs=1, space="PSUM") as accp,
            tc.tile_pool(name="wup", bufs=1, space="PSUM") as wup,
        ):
            b1sb = cst.tile([128, nks * napk * NPC], dt.float16, tag="b1")
            b2sb = cst.tile([128, NB * 2 * NPC], dt.float16, tag="b2")
            ssb = cst.tile([128, 128], dt.float16, tag="s2")
            biasb = cst.tile([NCLS, 1], dt.float32, tag="bias")
            lgsb = cst.tile([NCLS, NPC], dt.float32, tag="logits")

            nc.sync.dma_start(ssb[:, :], s2[:, :])
            for bi in range(4):
                nc.sync.dma_start(
                    b1sb[32 * bi : 32 * bi + 2, :], b1p[2 * bi : 2 * bi + 2, :]
                )
            nc.sync.dma_start(biasb[:, :], bia[:, :])

            if N_WARMUP:
                # keep the PE p-state warm while input DMAs land
                wups = wup.tile([64, NPC], dt.float32)
                for _ in range(N_WARMUP):
                    nc.tensor.matmul(
                        wups[:, :],
                        ssb[0:64, 0:64],
                        ssb[0:64, 0:NPC],
                        start=True,
                        stop=True,
                        skip_group_check=True,
                    )

            # two independent accumulation chains so the DMA-fed (C) and
            # PE-fed (A) pipelines never serialize each other
    
            acc_a = accp.tile([NCLS, NPC], dt.float32, tag="acc_a")
            acc_c = accp.tile([NCLS, NPC], dt.float32, tag="acc_c")
            routes = {}
            for (route, k, m0, size) in groups:
                for g in range(size):
                    routes[k * NM + m0 + g] = "A" if route == "A" else "C"
            a_chunks = [c for c in sorted(routes) if routes[c] == "A"]
            c_chunks = [c for c in sorted(routes) if routes[c] == "C"]
            bounds = {
                "A": (a_chunks[0], a_chunks[-1]),
                "C": (c_chunks[0], c_chunks[-1]),
            }
            ci = 0
            ci2 = 0
            chunk = 0
            apair = {k: 0 for k in range(NB)}
            wtiles = {}

            def stage1(route, k, m0, size):
                nonlocal ci, ci2
                if k not in wtiles:
                    # issue this k-block's W and tail tiles just-in-time so
                    # early route-C DMAs aren't queued behind all of W
                    nc.sync.dma_start(
                        b2sb[:, k * 2 * NPC : (k + 1) * 2 * NPC],
                        b2d[:, k * 2 * NPC : (k + 1) * 2 * NPC],
                    )
                    wk = wp.tile([128, NM * NCLS], dt.float16, tag="wk")
                    nc.sync.dma_start(wk[:, :], wt[k])
                    wtiles[k] = wk
                if route == "C":
                    r1sb = r1c.tile([128, GC * NPC], dt.float16, tag="r1c")
                    nc.sync.dma_start(r1sb[:, :], b1r[ci])
                    ci += 1
                elif route == "c":
                    r1sb = r1c.tile([128, GA * NPC], dt.float16, tag="r1c")
                    nc.sync.dma_start(r1sb[:, :], b1r2[ci2])
                    ci2 += 1
                else:
                    r1sb = r1a.tile([128, GA * NPC], dt.float16, tag="r1a")
                    b = (k % 4) * 32
                    for g in range(GA):
                        off = ((k // 4) * napk + apair[k]) * NPC
                        apair[k] += 1
                        r1ps = ps.tile([128, NPC], dt.float32, tag="r1ps")
                        nc.tensor.matmul(
                            r1ps[:, :],
                            ssb[b : b + 2, 0:128],
                            b1sb[b : b + 2, off : off + NPC],
                            start=True,
                            stop=True,
                            skip_group_check=True,
                            tile_position=(b, 0),
                        )
                        nc.scalar.copy(
                            r1sb[:, g * NPC : (g + 1) * NPC], r1ps[:, :]
                        )
                return r1sb

            def stage2(route, k, m0, size, r1sb, chunk0):
                pool = bltc if route in ("C", "c") else blta
                tag = "bltc" if route in ("C", "c") else "blta"
                blt = pool.tile([128, size * NPC], dt.float16, tag=tag)
                b2slice = b2sb[:, k * 2 * NPC : (k + 1) * 2 * NPC]
                for h in range(0, size, 2):
                    nc.vector.tensor_mul(
                        blt[:, h * NPC : (h + 2) * NPC],
                        r1sb[:, h * NPC : (h + 2) * NPC],
                        b2slice,
                    )
                wsb = wtiles[k]
                acc = acc_a if route == "A" else acc_c
                first, last = bounds["A" if route == "A" else "C"]
                for g in range(size):
                    c = chunk0 + g
                    cl = m0 + g
                    nc.tensor.matmul(
                        acc[:, :],
                        wsb[:, cl * NCLS : (cl + 1) * NCLS],
                        blt[:, g * NPC : (g + 1) * NPC],
                        start=(c == first),
                        stop=(c == last),
                        skip_group_check=True,
                    )

            SKEW = 1
            pending = []
            for gi, (route, k, m0, size) in enumerate(groups):
                r1sb = stage1(route, k, m0, size)
                pending.append((route, k, m0, size, r1sb, chunk))
                chunk += size
                if len(pending) > SKEW:
                    stage2(*pending.pop(0))
            for p in pending:
                stage2(*p)
            import concourse.mybir as _mybir
            acc_a_sb = cst.tile([NCLS, NPC], dt.float32, tag="acc_a_sb")
            nc.scalar.copy(acc_a_sb[:, :], acc_a[:, :])
            nc.vector.scalar_tensor_tensor(
                lgsb[:, :],
                acc_c[:, :],
                biasb[:, :],
                acc_a_sb[:, :],
                op0=_mybir.AluOpType.add,
                op1=_mybir.AluOpType.add,
            )
            nc.sync.dma_start(out[:, :], lgsb[:, :])

    _split_excess_waits(nc, limit=1)
    return nc


def _prep_shared(W, b):
    # W [97, 49152] -> wt [12, 128, 32*97] fp16; chunk (k, m) partition
    # p = di*64 + j corresponds to W[c, k, 2m+di, j].
    Wr = np.asarray(W, np.float32).reshape(NCLS, NB, NM, 2, BLK)
    wt = (
        Wr.transpose(3, 4, 1, 2, 0)  # [di, j, k, m, c]
        .reshape(128, NB, NM * NCLS)
        .transpose(1, 0, 2)
        .astype(np.float16)
    )
    bias = np.asarray(b, np.float32).reshape(NCLS, 1)
    # s2[b+q, p] = 1 iff q == p//64, for each base b in {0,32,64,96}
    s2 = np.zeros((128, 128), np.float16)
    for base in (0, 32, 64, 96):
        s2[base, :64] = 1.0
        s2[base + 1, 64:] = 1.0
    return np.ascontiguousarray(wt), bias, s2


def _prep_core(head, tail, groups):
    b1T = np.asarray(head, np.float32).T.astype(np.float16)  # [768, NPC]
    napk = sum(2 for r in K_PATTERNS[0] if r == "A")
    nks = (NB + 3) // 4
    b1p = np.zeros((8, nks * napk * NPC), np.float16)
    apair = {k: 0 for k in range(NB)}
    for (route, k, m0, size) in groups:
        if route != "A":
            continue
        bi = k % 4
        for g in range(size):
            m = m0 + g
            s = (k // 4) * napk + apair[k]
            apair[k] += 1
            for d in (0, 1):
                b1p[2 * bi + d, s * NPC : (s + 1) * NPC] = b1T[64 * k + 2 * m + d]
    b2T = np.asarray(tail, np.float32).T.astype(np.float16).reshape(NB, BLK, NPC)
    b2dup = np.concatenate([b2T, b2T], axis=1)  # [12, 128, NPC]
    b2d = (
        np.broadcast_to(b2dup[:, None], (NB, 2, 128, NPC))
        .transpose(2, 0, 1, 3)
        .reshape(128, NB * 2 * NPC)
    )
    # host-replicated groups: b1r[gi, p, g*NPC + n] = b1T[64k + 2(m0+g) + p//64, n]
    pairs = b1T.reshape(NB, NM, 2, NPC)
    b1r_list, b1r2_list = [], []
    for (route, k, m0, size) in groups:
        if route == "A":
            continue
        sel = pairs[k, m0 : m0 + size]          # [size, 2, NPC]
        rep = np.repeat(sel, 64, axis=1)        # [size, 128, NPC]
        arr = rep.transpose(1, 0, 2).reshape(128, size * NPC)
        (b1r_list if route == "C" else b1r2_list).append(arr)
    b1r = np.stack(b1r_list, axis=0)
    if b1r2_list:
        b1r2 = np.stack(b1r2_list, axis=0)
    else:
        b1r2 = np.zeros((1, 128, GA * NPC), np.float16)
    return (
        b1p,
        np.ascontiguousarray(b2d),
        np.ascontiguousarray(b1r),
        np.ascontiguousarray(b1r2),
    )


def kernel(head_embeddings, tail_embeddings, W, b):
    from concourse.bass_utils import run_bass_kernel_spmd

    assert head_embeddings.shape == (NTOT, EMB), head_embeddings.shape
    assert tail_embeddings.shape == (NTOT, EMB), tail_embeddings.shape
    assert W.shape == (NCLS, EMB * BLK), W.shape

    if "nc" not in _CACHE:
        _CACHE["nc"] = _build_nc()
    nc = _CACHE["nc"]

    groups = _groups()
    wt, bias, s2 = _prep_shared(W, b)
    in_maps = []
    for i in range(NCORES):
        s = slice(i * NPC, (i + 1) * NPC)
        b1p, b2d, b1r, b1r2 = _prep_core(
            head_embeddings[s], tail_embeddings[s], groups
        )
        in_maps.append(
            {
                "b1p": b1p,
                "b2d": b2d,
                "wt": wt,
                "bias": bias,
                "s2": s2,
                "b1r": b1r,
                "b1r2": b1r2,
            }
        )

    res = run_bass_kernel_spmd(nc, in_maps, list(range(NCORES)))
    _CACHE["last_results"] = res
    logits = np.concatenate(
        [res.results[i]["logits_t"].T for i in range(NCORES)], axis=0
    )
    return logits.astype(np.float32)



# revision 3
# speedup vs baseline: 1.1988x; 1.1988x over previous
"""Trainium2 Bass kernel for the bilinear block classifier.

logits[n, c] = sum_{k,i,j} W[c, k*4096+i*64+j] * head[n, 64k+i] * tail[n, 64k+j] + b[c]
head/tail [4096, 768] fp32, W [97, 49152] fp32, b [97] fp32.

Data-parallel over 8 NeuronCores (512 samples each). Per block k (12 blocks
of 64x64 outer products), the 128 partitions carry a 16x8 (i0, j0) split of
the (i, j) feature space; the remaining 4x8 (i1, j1) combinations unroll on
the free dim of a single elementwise multiply whose inputs are read with
stride-0 free-dim repeats:

    feat[(i0,j0), (i1,j1,n)] = h[k, 4*i0+i1, n] * t[k, 8*j0+j1, n]
      in0 AP [[.,128],[512,4],[0,8],[1,512]]   (h-tile, 8x j1-repeat, free)
      in1 AP [[.,128],[0,4],[512,8],[1,512]]   (t-tile, 4x i1-repeat, free)

so only 12x- (vs 64x-) redundant tiles are shipped from HBM (19 MB vs 50 MB).
The multiply is split between VectorE (2x fp16 mode) and GPSIMD per a static
plan. Stage 2 contracts each 128-feature chunk against W with the feature
tile as the stationary matmul operand: out[128 samples, 97 classes], 97-row
cost per matmul, fp32 PSUM accumulation across all 384 chunks. Bias enters
as a ones-vector matmul against a replicated bias tile. Output is the
natural [samples, classes] layout.
"""

import numpy as np

EMB = 768
BLK = 64
NCLS = 97
NTOT = 4096
NB = 12             # feature blocks of 64x64
NCORES = 8
NPC = NTOT // NCORES    # 512 samples per core
I0, I1 = 16, 4      # i = 4*i0 + i1
J0, J1 = 8, 8       # j = 8*j0 + j1
NW = 4              # sample windows of 128 (stage-2 output partitions)
WIN = NPC // NW

# multiply plan per block: list of (engine, j1_start, j1_count)
# "v" = VectorE (DVE), "p" = GPSIMD (Pool)
_POOL_COLS = [2, 2, 2, 2, 2, 2, 2, 2, 1, 1, 1, 1]   # 20 cols -> 80 chunks


def _segments(k):
    pc = _POOL_COLS[k]
    vc = J1 - pc
    segs = []
    h = vc // 2
    if h:
        segs.append(("v", 0, h))
    if vc - h:
        segs.append(("v", h, vc - h))
    if pc:
        segs.append(("p", vc, pc))
    return segs


_CACHE = {}


def _split_excess_waits(nc, limit=1):
    """walrus in this toolchain rejects instructions carrying more than
    `limit` semaphore waits; split extras into preceding wait-only Drains."""
    import concourse.mybir as mybir

    n_new = 0
    for bb in nc.main_func.blocks:
        new_list = []
        for ins in bb.instructions:
            si = ins.sync_info
            if si is not None and si.on_wait and len(si.on_wait) > limit:
                waits = list(si.on_wait)
                extra, keep = waits[:-limit], waits[-limit:]
                for i in range(0, len(extra), limit):
                    chunk = extra[i : i + limit]
                    n_new += 1
                    d = mybir.InstDrain(
                        name=f"I-waitsplit-{n_new}",
                        engine=ins.engine,
                        ins=[],
                        outs=[],
                        sync_info=mybir.SyncInfo(on_wait=chunk, on_update=[]),
                    )
                    nc.register_instruction(d)
                    new_list.append(d)
                si.on_wait = keep
            new_list.append(ins)
        bb.instructions[:] = new_list
    return n_new


def _build_nc():
    import concourse.bass as bass
    import concourse.mybir as mybir
    import concourse.tile as tile
    from concourse.ap import AP

    dt = mybir.dt
    nc = bass.Bass()

    ht_d = nc.dram_tensor("ht", [NB, 128, I1 * NPC], dt.float16, kind="ExternalInput")
    tt_d = nc.dram_tensor("tt", [NB, 128, J1 * NPC], dt.float16, kind="ExternalInput")
    wt_d = nc.dram_tensor("wt", [NB, 128, I1 * J1 * NCLS], dt.float16,
                          kind="ExternalInput")
    br_d = nc.dram_tensor("br", [128, NCLS], dt.float16, kind="ExternalInput")
    out_d = nc.dram_tensor("out", [128, NW * NCLS], dt.float32, kind="ExternalOutput")

    with tile.TileContext(nc) as tc:
        with (
            tc.tile_pool(name="cst", bufs=1) as cst,
            tc.tile_pool(name="hp", bufs=2) as hp,
            tc.tile_pool(name="tp", bufs=2) as tp,
            tc.tile_pool(name="wp", bufs=2) as wp,
            tc.tile_pool(name="fp", bufs=2) as fp,
            tc.tile_pool(name="accp", bufs=1, space="PSUM") as accp,
        ):
            ones = cst.tile([128, 128], dt.float16, tag="ones")
            brsb = cst.tile([128, NCLS], dt.float16, tag="br")
            lg = cst.tile([128, NW * NCLS], dt.float32, tag="lg")

            nc.sync.dma_start(brsb[:, :], br_d[:, :])
            nc.vector.memset(ones[:, :], 1.0)

            accs = []
            for w in range(NW):
                acc = accp.tile([128, NCLS], dt.float32, tag=f"acc{w}")
                accs.append(acc)
            # open each window's accumulation group with the bias term:
            # sum_p ones[p, s] * (b[c]/128) = b[c]
            for w in range(NW):
                nc.tensor.matmul(
                    accs[w][:, :], ones[:, :], brsb[:, :],
                    start=True, stop=False, skip_group_check=True,
                )

            for k in range(NB):
                hk = hp.tile([128, I1 * NPC], dt.float16, tag="hk")
                tk = tp.tile([128, J1 * NPC], dt.float16, tag="tk")
                wk = wp.tile([128, I1 * J1 * NCLS], dt.float16, tag="wk")
                nc.sync.dma_start(hk[:, :], ht_d[k])
                nc.sync.dma_start(tk[:, :], tt_d[k])
                nc.sync.dma_start(wk[:, :], wt_d[k])

                fk = fp.tile([128, I1 * J1 * NPC], dt.float16, tag="fk")
                f_ap = fk[:, :]
                h_ap = hk[:, :]
                t_ap = tk[:, :]
                for (eng, j1s, j1c) in _segments(k):
                    out_ap = AP(f_ap.tensor, f_ap.offset + j1s * NPC,
                                [list(f_ap.ap[0]),
                                 [J1 * NPC, I1], [NPC, j1c], [1, NPC]])
                    in_h = AP(h_ap.tensor, h_ap.offset,
                              [list(h_ap.ap[0]),
                               [NPC, I1], [0, j1c], [1, NPC]])
                    in_t = AP(t_ap.tensor, t_ap.offset + j1s * NPC,
                              [list(t_ap.ap[0]),
                               [0, I1], [NPC, j1c], [1, NPC]])
                    e = nc.vector if eng == "v" else nc.gpsimd
                    e.tensor_mul(out_ap, in_h, in_t)

                last_k = k == NB - 1
                for i1 in range(I1):
                    for j1 in range(J1):
                        ch = i1 * J1 + j1
                        last_ch = last_k and i1 == I1 - 1 and j1 == J1 - 1
                        for w in range(NW):
                            nc.tensor.matmul(
                                accs[w][:, :],
                                fk[:, ch * NPC + w * WIN : ch * NPC + (w + 1) * WIN],
                                wk[:, ch * NCLS : (ch + 1) * NCLS],
                                start=False, stop=last_ch,
                                skip_group_check=True,
                            )

            for w in range(NW):
                nc.scalar.copy(lg[:, w * NCLS : (w + 1) * NCLS], accs[w][:, :])
            nc.sync.dma_start(out_d[:, :], lg[:, :])

    _split_excess_waits(nc, limit=1)
    return nc


def _prep_shared(W, b):
    # wt[k, i0*8+j0, (i1*8+j1)*97 + c] = W[c, k, 4*i0+i1, 8*j0+j1]
    Wr = np.asarray(W, np.float32).reshape(NCLS, NB, I0, I1, J0, J1)
    wt = (
        Wr.transpose(1, 2, 4, 3, 5, 0)      # k, i0, j0, i1, j1, c
        .reshape(NB, 128, I1 * J1 * NCLS)
        .astype(np.float16)
    )
    br = np.broadcast_to(
        (np.asarray(b, np.float32) / 128.0).astype(np.float16)[None, :],
        (128, NCLS),
    )
    return np.ascontiguousarray(wt), np.ascontiguousarray(br)


def _prep_core(head, tail):
    hT = np.asarray(head, np.float32).T.astype(np.float16)  # [768, NPC]
    tT = np.asarray(tail, np.float32).T.astype(np.float16)
    # ht[k, i0*8+j0, i1*NPC+n] = hT[64k+4*i0+i1, n]
    hblk = hT.reshape(NB, I0, I1, NPC)
    ht = np.broadcast_to(
        hblk[:, :, None, :, :], (NB, I0, J0, I1, NPC)
    ).reshape(NB, 128, I1 * NPC)
    # tt[k, i0*8+j0, j1*NPC+n] = tT[64k+8*j0+j1, n]
    tblk = tT.reshape(NB, J0, J1, NPC)
    tt = np.broadcast_to(
        tblk[:, None, :, :, :], (NB, I0, J0, J1, NPC)
    ).reshape(NB, 128, J1 * NPC)
    return np.ascontiguousarray(ht), np.ascontiguousarray(tt)


def kernel(head_embeddings, tail_embeddings, W, b):
    from concourse.bass_utils import run_bass_kernel_spmd

    assert head_embeddings.shape == (NTOT, EMB), head_embeddings.shape
    assert tail_embeddings.shape == (NTOT, EMB), tail_embeddings.shape
    assert W.shape == (NCLS, EMB * BLK), W.shape

    if "nc" not in _CACHE:
        _CACHE["nc"] = _build_nc()
    nc = _CACHE["nc"]

    wt, br = _prep_shared(W, b)
    in_maps = []
    for i in range(NCORES):
        s = slice(i * NPC, (i + 1) * NPC)
        ht, tt = _prep_core(head_embeddings[s], tail_embeddings[s])
        in_maps.append({"ht": ht, "tt": tt, "wt": wt, "br": br})

    res = run_bass_kernel_spmd(nc, in_maps, list(range(NCORES)))
    _CACHE["last_results"] = res
    # out[s, w*97+c] -> logits rows w*128+s
    logits = np.concatenate(
        [
            res.results[i]["out"].reshape(128, NW, NCLS)
            .transpose(1, 0, 2).reshape(NPC, NCLS)
            for i in range(NCORES)
        ],
        axis=0,
    )
    return logits.astype(np.float32)


# revision 18
# speedup vs baseline: 1.4504x; 1.2099x over previous
"""Trainium2 Bass kernel for the bilinear block classifier.

logits[n, c] = sum_{k,i,j} W[c, k*4096+i*64+j] * head[n, 64k+i] * tail[n, 64k+j] + b[c]
head/tail [4096, 768] fp32, W [97, 49152] fp32, b [97] fp32.

Data-parallel over 8 NeuronCores (512 samples each). Per block k (12 blocks
of 64x64 outer products) the feature space is covered by three producer
routes, all writing fp16 feature chunks consumed by a uniform stage-2:

  D (VectorE): partitions carry a 16x8 (i0, j0) split; the remaining
     4 x j1-columns unroll on the free dim of one tensor multiply whose
     inputs use stride-0 free-dim repeats, so only 12x-redundant h/t tiles
     ship from HBM (vs 64x for naive partition replication).
  P (GPSIMD): same structure, trailing j1-columns, on the Pool engine.
     Its stage-2 matmuls are deferred by a fixed block lag so the slower
     engine never stalls the pipeline.
  S (square): feat = h*t = ((h+t)^2 - h^2 - t^2)/2. A PE selection matmul
     builds s = h_i + t_j replicated across the chunk's partitions from a
     compact raw tile; ScalarE evacuates Square(s/sqrt2) = s^2/2 straight
     into the feature slice. The -h^2/2, -t^2/2 terms collapse into one
     correction chunk per block whose weights are host-side row/col sums
     of W over the S-columns.

Stage 2 contracts each 128-feature chunk against W with the feature tile
stationary: out[128 samples, 97 classes] costs 97 PE rows per matmul, fp32
PSUM accumulation across all chunks; bias enters as a ones-vector matmul.
Output is the natural [samples, classes] layout.
"""

import numpy as np

EMB = 768
BLK = 64
NCLS = 97
NTOT = 4096
NB = 12             # feature blocks of 64x64
NCORES = 8
NPC = NTOT // NCORES    # 512 samples per core
I0, I1 = 16, 4      # i = 4*i0 + i1
J0, J1 = 8, 8       # j = 8*j0 + j1
NW = 4              # sample windows of 128 (stage-2 output partitions)
WIN = NPC // NW
LAG = 4             # blocks of slack granted to the GPSIMD route

# per-block column plan: of the 8 j1-columns, the first DC go to VectorE,
# the next PC to GPSIMD, and column 7 to the square route when SC == 1.
S_COLS = [[], [7], [7], [7], [7], [7], [7], [7], [7], [7], [7], [7]]
SC = [len(S_COLS[k]) for k in range(NB)]
PC = [2, 2, 2, 2, 2, 1, 1, 1, 1, 1, 1, 1]           # GPSIMD cols
DC = [8 - SC[k] - PC[k] for k in range(NB)]
S_BLOCKS = [k for k in range(NB) if SC[k]]
NSB = len(S_BLOCKS)
NSCHUNK = sum(SC)
SEL_J1S = sorted({j for cols in S_COLS for j in cols})
WSCALE = 512.0
MC1 = max(DC[k] + SC[k] for k in range(NB))          # wt1 col capacity
MC2 = max(PC)                                        # wt2 col capacity

_CACHE = {}


def _split_excess_waits(nc, limit=1):
    """walrus in this toolchain rejects instructions carrying more than
    `limit` semaphore waits; split extras into preceding wait-only Drains."""
    import concourse.mybir as mybir

    n_new = 0
    for bb in nc.main_func.blocks:
        new_list = []
        for ins in bb.instructions:
            si = ins.sync_info
            if si is not None and si.on_wait and len(si.on_wait) > limit:
                waits = list(si.on_wait)
                extra, keep = waits[:-limit], waits[-limit:]
                for i in range(0, len(extra), limit):
                    chunk = extra[i : i + limit]
                    n_new += 1
                    d = mybir.InstDrain(
                        name=f"I-waitsplit-{n_new}",
                        engine=ins.engine,
                        ins=[],
                        outs=[],
                        sync_info=mybir.SyncInfo(on_wait=chunk, on_update=[]),
                    )
                    nc.register_instruction(d)
                    new_list.append(d)
                si.on_wait = keep
            new_list.append(ins)
        bb.instructions[:] = new_list
    return n_new


def _build_nc():
    import concourse.bass as bass
    import concourse.mybir as mybir
    import concourse.tile as tile
    from concourse.ap import AP

    dt = mybir.dt
    nc = bass.Bass()

    ht_d = nc.dram_tensor("ht", [NB, 128, I1 * NPC], dt.float16, kind="ExternalInput")
    tt_d = nc.dram_tensor("tt", [NB, 128, J1 * NPC], dt.float16, kind="ExternalInput")
    wt1_d = nc.dram_tensor("wt1", [NB, 128, MC1 * I1 * NCLS], dt.float8e3,
                           kind="ExternalInput")
    wt2_d = nc.dram_tensor("wt2", [NB, 128, MC2 * I1 * NCLS], dt.float8e3,
                           kind="ExternalInput")
    br_d = nc.dram_tensor("br", [128, NCLS], dt.float16, kind="ExternalInput")
    sel_d = nc.dram_tensor("sel", [128, len(SEL_J1S) * I1 * 128], dt.float16,
                       kind="ExternalInput")
    raw_d = nc.dram_tensor("raw", [NB, 128, NPC], dt.float16, kind="ExternalInput")
    wc_d = nc.dram_tensor("wc", [128, NSB * NCLS], dt.float16, kind="ExternalInput")
    out_d = nc.dram_tensor("out", [128, NW * NCLS], dt.float32, kind="ExternalOutput")

    with tile.TileContext(nc) as tc:
        with (
            tc.tile_pool(name="cst", bufs=1) as cst,
            tc.tile_pool(name="hp", bufs=LAG + 3) as hp,
            tc.tile_pool(name="tp", bufs=4) as tp,
            tc.tile_pool(name="tp2", bufs=LAG + 2) as tp2,
            tc.tile_pool(name="wp", bufs=3) as wp,
            tc.tile_pool(name="wp2", bufs=LAG + 2) as wp2,
            tc.tile_pool(name="fp", bufs=2) as fp,
            tc.tile_pool(name="fpp", bufs=LAG + 2) as fpp,
            tc.tile_pool(name="rawp", bufs=4) as rawp,
            tc.tile_pool(name="sqp", bufs=2) as sqp,
            tc.tile_pool(name="accp", bufs=1, space="PSUM") as accp,
            tc.tile_pool(name="psp", bufs=4, space="PSUM") as psp,
        ):
            ones = cst.tile([128, 128], dt.float16, tag="ones")
            brsb = cst.tile([128, NCLS], dt.float16, tag="br")
            selsb = cst.tile([128, len(SEL_J1S) * I1 * 128], dt.float16, tag="sel")
            wcsb = cst.tile([128, NSB * NCLS], dt.float16, tag="wc")
            lg = cst.tile([128, NW * NCLS], dt.float32, tag="lg")

            accs = []
            for w in range(NW):
                acc = accp.tile([128, NCLS], dt.float32, tag=f"acc{w}")
                accs.append(acc)

            # DMA program (SP queue is in-order): block-0 tiles first for a
            # short pipeline head, then constants, then the block stream.
            hks, tks, tk2s, wks, wk2s, raws = {}, {}, {}, {}, {}, {}

            def issue_data(k, first=False):
                hk = hp.tile([128, I1 * NPC], dt.float16, tag="hk")
                nc.sync.dma_start(hk[:, :], ht_d[k])
                hks[k] = hk
                dc, pc, sc = DC[k], PC[k], SC[k]
                tk = tp.tile([128, dc * NPC], dt.float16, tag="tk")
                if first:
                    # split so the first multiply segment starts earlier
                    nc.sync.dma_start(tk[:, 0 : 2 * NPC],
                                      tt_d[k][:, 0 : 2 * NPC])
                    nc.sync.dma_start(tk[:, 2 * NPC : dc * NPC],
                                      tt_d[k][:, 2 * NPC : dc * NPC])
                else:
                    nc.sync.dma_start(tk[:, :], tt_d[k][:, 0 : dc * NPC])
                tks[k] = tk
                if pc:
                    tk2 = tp2.tile([128, pc * NPC], dt.float16, tag="tk2")
                    nc.sync.dma_start(
                        tk2[:, :], tt_d[k][:, dc * NPC : (dc + pc) * NPC])
                    tk2s[k] = tk2
                if sc:
                    raw = rawp.tile([128, NPC], dt.float16, tag="raw")
                    nc.sync.dma_start(raw[:, :], raw_d[k])
                    raws[k] = raw

            def issue_w(k):
                dc, pc, sc = DC[k], PC[k], SC[k]
                wk = wp.tile([128, (dc + sc) * I1 * NCLS], dt.float8e3, tag="wk")
                nc.sync.dma_start(
                    wk[:, :], wt1_d[k][:, 0 : (dc + sc) * I1 * NCLS])
                wks[k] = wk
                if pc:
                    wk2 = wp2.tile([128, pc * I1 * NCLS], dt.float8e3, tag="wk2")
                    nc.sync.dma_start(
                        wk2[:, :], wt2_d[k][:, 0 : pc * I1 * NCLS])
                    wk2s[k] = wk2

            nc.sync.dma_start(brsb[:, :], br_d[:, :])
            nc.sync.dma_start(selsb[:, :], sel_d[:, :])
            nc.sync.dma_start(wcsb[:, :], wc_d[:, :])
            nc.vector.memset(ones[:, :], 1.0)
            issue_data(0, first=True)
            issue_w(0)
            issue_data(1)
            issue_w(1)
            issue_data(2)

            # open each window's accumulation group with the bias term:
            # sum_p ones[p, s] * (b[c]/128) = b[c]
            for w in range(NW):
                nc.tensor.matmul(
                    accs[w][:, :], ones[:, :], brsb[:, :],
                    start=True, stop=False, skip_group_check=True,
                )

            def stage2(fk_ap, w_ap, last=False):
                for w in range(NW):
                    nc.tensor.matmul(
                        accs[w][:, :],
                        fk_ap[:, w * WIN : (w + 1) * WIN],
                        w_ap,
                        start=False, stop=last,
                        skip_group_check=True,
                    )

            pool_work = []   # deferred stage-2 for GPSIMD-produced chunks

            def emit_pool_stage2(last_blk=False):
                (k, fkp, wk2) = pool_work.pop(0)
                pc = PC[k]
                for q in range(pc):
                    for i1 in range(I1):
                        last = last_blk and q == pc - 1 and i1 == I1 - 1
                        stage2(fkp[:, (i1 * pc + q) * NPC
                                   : (i1 * pc + q + 1) * NPC],
                               wk2[:, (i1 * pc + q) * NCLS
                                   : (i1 * pc + q + 1) * NCLS],
                               last)

            for k in range(NB):
                if k + 2 < NB:
                    issue_w(k + 2)
                if k + 3 < NB:
                    issue_data(k + 3)
                hk, tk, wk = hks[k], tks[k], wks[k]
                dc, pc, sc = DC[k], PC[k], SC[k]
                ncol = dc + sc

                # --- S route: sel-matmul sums + Act squares into fk ---
                fk = fp.tile([128, ncol * I1 * NPC], dt.float16, tag="fk")
                f_ap = fk[:, :]
                if sc:
                    raw = raws[k]
                    sq = sqp.tile([128, NPC], dt.float16, tag="sq")
                    nc.scalar.activation(
                        sq[:, :], raw[:, :],
                        mybir.ActivationFunctionType.Square, 0.0, 1.0, 0.0)
                    for sc_i, j1v in enumerate(S_COLS[k]):
                        soff = SEL_J1S.index(j1v) * I1 * 128
                        for i1 in range(I1):
                            ps = psp.tile([128, NPC], dt.float32, tag="ps")
                            nc.tensor.matmul(
                                ps[:, :],
                                selsb[:, soff + i1 * 128 : soff + (i1 + 1) * 128],
                                raw[:, :],
                                start=True, stop=True, skip_group_check=True)
                            nc.scalar.activation(
                                fk[:, (i1 * ncol + dc + sc_i) * NPC
                                   : (i1 * ncol + dc + sc_i + 1) * NPC],
                                ps[:, :],
                                mybir.ActivationFunctionType.Square,
                                0.0, 0.7071067811865476, 0.0)

                # --- D route: VectorE multiply, split into two segments ---
                h_ap = hk[:, :]
                t_ap = tk[:, :]
                if k == 0:
                    segs = [(0, 2), (2, dc - 2)]
                elif k == NB - 1:
                    segs = [(0, dc - 1), (dc - 1, 1)]
                else:
                    h2 = (dc + 1) // 2
                    segs = [(0, h2), (h2, dc - h2)]
                for (j1s, j1c) in segs:
                    if not j1c:
                        continue
                    out_ap = AP(f_ap.tensor, f_ap.offset + j1s * NPC,
                                [list(f_ap.ap[0]),
                                 [ncol * NPC, I1], [NPC, j1c], [1, NPC]])
                    in_h = AP(h_ap.tensor, h_ap.offset,
                              [list(h_ap.ap[0]),
                               [NPC, I1], [0, j1c], [1, NPC]])
                    in_t = AP(t_ap.tensor, t_ap.offset + j1s * NPC,
                              [list(t_ap.ap[0]),
                               [0, I1], [NPC, j1c], [1, NPC]])
                    nc.vector.tensor_mul(out_ap, in_h, in_t)

                # --- P route: GPSIMD multiply into its own tile ---
                if pc:
                    tk2 = tk2s[k]
                    fkp = fpp.tile([128, pc * I1 * NPC], dt.float16, tag="fkp")
                    fp_ap = fkp[:, :]
                    t2_ap = tk2[:, :]
                    out_ap = AP(fp_ap.tensor, fp_ap.offset,
                                [list(fp_ap.ap[0]),
                                 [pc * NPC, I1], [NPC, pc], [1, NPC]])
                    in_h = AP(h_ap.tensor, h_ap.offset,
                              [list(h_ap.ap[0]),
                               [NPC, I1], [0, pc], [1, NPC]])
                    in_t = AP(t2_ap.tensor, t2_ap.offset,
                              [list(t2_ap.ap[0]),
                               [0, I1], [NPC, pc], [1, NPC]])
                    nc.gpsimd.tensor_mul(out_ap, in_h, in_t)
                    pool_work.append((k, fkp, wk2s[k]))

                # --- deferred pool stage-2 from LAG blocks ago ---
                if k >= LAG and pool_work:
                    emit_pool_stage2()

                # --- stage 2 for D then S chunks of this block ---
                for q in range(dc):
                    for i1 in range(I1):
                        stage2(fk[:, (i1 * ncol + q) * NPC
                                  : (i1 * ncol + q + 1) * NPC],
                               wk[:, (i1 * ncol + q) * NCLS
                                  : (i1 * ncol + q + 1) * NCLS])
                if sc:
                    for sc_i in range(sc):
                        for i1 in range(I1):
                            stage2(fk[:, (i1 * ncol + dc + sc_i) * NPC
                                      : (i1 * ncol + dc + sc_i + 1) * NPC],
                                   wk[:, (i1 * ncol + dc + sc_i) * NCLS
                                      : (i1 * ncol + dc + sc_i + 1) * NCLS])
                    # correction chunk: sq features vs summed weights
                    si = S_BLOCKS.index(k)
                    stage2(sq[:, :], wcsb[:, si * NCLS : (si + 1) * NCLS])

            while pool_work:
                emit_pool_stage2(last_blk=len(pool_work) == 1)

            for w in range(NW):
                nc.scalar.mul(lg[:, w * NCLS : (w + 1) * NCLS], accs[w][:, :],
                              1.0 / WSCALE)
            nc.sync.dma_start(out_d[:, :], lg[:, :])

    _split_excess_waits(nc, limit=1)
    return nc


def _prep_shared(W, b):
    import ml_dtypes
    # Wv[c, k, i1, j1, p] with p = i0*8+j0
    Wr = (np.asarray(W, np.float32) * WSCALE).reshape(NCLS, NB, I0, I1, J0, J1)
    Wv = Wr.transpose(1, 2, 4, 3, 5, 0)     # k, i0, j0, i1, j1, c
    Wv = Wv.reshape(NB, 128, I1, J1, NCLS)
    wt1 = np.zeros((NB, 128, MC1 * I1 * NCLS), ml_dtypes.float8_e3m4)
    wt2 = np.zeros((NB, 128, MC2 * I1 * NCLS), ml_dtypes.float8_e3m4)
    for k in range(NB):
        dc, pc, sc = DC[k], PC[k], SC[k]
        cols1 = list(range(dc)) + list(S_COLS[k])
        w1 = Wv[k][:, :, cols1, :]          # [128, I1, ncol, NCLS]
        wt1[k, :, : (dc + sc) * I1 * NCLS] = (
            w1.reshape(128, -1).astype(ml_dtypes.float8_e3m4))
        cols2 = list(range(dc, dc + pc))
        w2 = Wv[k][:, :, cols2, :]
        wt2[k, :, : pc * I1 * NCLS] = w2.reshape(128, -1).astype(
            ml_dtypes.float8_e3m4)
    br = np.broadcast_to(
        (np.asarray(b, np.float32) * (WSCALE / 128.0)).astype(np.float16)[None, :],
        (128, NCLS),
    )
    # selection matrices for the square route:
    # sel[kk, (jv, i1, p)], p = i0*8+j0: kk=4*i0+i1 -> 1, kk=64+8*j0+j1v -> 1
    sel = np.zeros((128, len(SEL_J1S) * I1 * 128), np.float16)
    for jx, j1v in enumerate(SEL_J1S):
        for i1 in range(I1):
            for i0 in range(I0):
                for j0 in range(J0):
                    p = i0 * J0 + j0
                    col = jx * I1 * 128 + i1 * 128 + p
                    sel[4 * i0 + i1, col] = 1.0
                    sel[64 + 8 * j0 + j1v, col] = 1.0
    # correction weights: -1/2 row/col sums of W over the S columns (j%8==7)
    Wb = (np.asarray(W, np.float32) * WSCALE).reshape(NCLS, NB, BLK, BLK)
    wc = np.zeros((128, NSB * NCLS), np.float32)
    for si, k in enumerate(S_BLOCKS):
        jmask = np.zeros(BLK, bool)
        for j1v in S_COLS[k]:
            jmask[j1v::8] = True
        wh = -0.5 * Wb[:, k][:, :, jmask].sum(axis=2)    # [NCLS, 64] over j in S
        wtc = -0.5 * Wb[:, k, :, :].sum(axis=1)          # [NCLS, 64] over all i
        wc[0:64, si * NCLS : (si + 1) * NCLS] = wh.T
        block = np.zeros((64, NCLS), np.float32)
        block[jmask, :] = wtc[:, jmask].T
        wc[64:128, si * NCLS : (si + 1) * NCLS] = block
    return (wt1, wt2, np.ascontiguousarray(br), sel, wc.astype(np.float16))


def _prep_core(head, tail):
    hT = np.asarray(head, np.float32).T.astype(np.float16)  # [768, NPC]
    tT = np.asarray(tail, np.float32).T.astype(np.float16)
    # ht[k, i0*8+j0, i1*NPC+n] = hT[64k+4*i0+i1, n]
    hblk = hT.reshape(NB, I0, I1, NPC)
    ht = np.broadcast_to(
        hblk[:, :, None, :, :], (NB, I0, J0, I1, NPC)
    ).reshape(NB, 128, I1 * NPC)
    # tt[k, i0*8+j0, j1*NPC+n] = tT[64k+8*j0+j1, n]
    tblk = tT.reshape(NB, J0, J1, NPC)
    tt = np.broadcast_to(
        tblk[:, None, :, :, :], (NB, I0, J0, J1, NPC)
    ).reshape(NB, 128, J1 * NPC)
    # raw[k]: rows 0..63 = h rows of block k, 64..127 = t rows
    raw = np.concatenate(
        [hT.reshape(NB, BLK, NPC), tT.reshape(NB, BLK, NPC)], axis=1
    )
    return (np.ascontiguousarray(ht), np.ascontiguousarray(tt),
            np.ascontiguousarray(raw))


def kernel(head_embeddings, tail_embeddings, W, b):
    from concourse.bass_utils import run_bass_kernel_spmd

    assert head_embeddings.shape == (NTOT, EMB), head_embeddings.shape
    assert tail_embeddings.shape == (NTOT, EMB), tail_embeddings.shape
    assert W.shape == (NCLS, EMB * BLK), W.shape

    if "nc" not in _CACHE:
        _CACHE["nc"] = _build_nc()
    nc = _CACHE["nc"]

    wt1, wt2, br, sel, wc = _prep_shared(W, b)
    in_maps = []
    for i in range(NCORES):
        s = slice(i * NPC, (i + 1) * NPC)
        ht, tt, raw = _prep_core(head_embeddings[s], tail_embeddings[s])
        in_maps.append({"ht": ht, "tt": tt, "wt1": wt1, "wt2": wt2, "br": br,
                        "sel": sel, "raw": raw, "wc": wc})

    res = run_bass_kernel_spmd(nc, in_maps, list(range(NCORES)))
    _CACHE["last_results"] = res
    # out[s, w*97+c] -> logits rows w*128+s
    logits = np.concatenate(
        [
            res.results[i]["out"].reshape(128, NW, NCLS)
            .transpose(1, 0, 2).reshape(NPC, NCLS)
            for i in range(NCORES)
        ],
        axis=0,
    )
    return logits.astype(np.float32)
